# revision 2
# baseline (speedup 1.0000x reference)
"""DualGCN Trainium2 kernel: 8-core SPMD via bass/Tile.

Strategy (per spec sharding_hint): nodes row-sharded across 8 cores.
Each GCNConv is computed as
    agg[d] = dinv[d] * ( sum_e w_e * htil[src_e] ) + bias,   htil = dinv .* (x @ W)
with self-loops appended as ordinary edges of weight 1 (so deg = rowsum of
edge weights and the self-loop term folds into the edge sum).

Per conv: row-sharded matmul on the TensorEngine (fp32r), dinv row-scale,
AllGather of htil across the 8 cores, then an ELL-style pass loop: pass k
gathers htil[src of k-th incoming edge] for every destination slot via
dma_gather and a fused scalar_tensor_tensor accumulates acc += w_k * G_k.
Destinations are locally sorted by in-degree so pass k only covers a prefix
of slots (minimal padding). Outputs are realigned to natural row order with
a unique-index dma_scatter_add into pre-zeroed DRAM outputs.

All numerical work (matmuls, degree sums, rsqrt, scaling, aggregation) is
done on device; the host only reorders/relabels indices and pads layouts.
"""
import sys

if '/opt/trn_rl_repo' not in sys.path:
    sys.path.insert(0, '/opt/trn_rl_repo')

import numpy as np

N = 20000
NCORES = 8
NLOC = N // NCORES            # 2500
CH = 20                       # chunks of 128 per core
NLOC_PAD = CH * 128           # 2560
IN_C = 2000
KPAD = 2048
HID = 512
OUT = 256
Q = 100
QPAD = 128
SUBBLK = 8                    # gather granularity: dma_gather tops out at 1024 idxs/op


def _wrap_idx(a):
    """Linear int array (len % 16 == 0) -> [128, len/16] int16 SWDGE layout."""
    t = a.reshape(-1, 16).T.astype(np.int16)
    return np.ascontiguousarray(np.tile(t, (8, 1)))


def _graph_prep(edge_index, edge_weight):
    """Per-graph host-side index preprocessing.

    Returns dict with per-core ELL structures:
      perm[c]      : [NLOC] slot -> natural local node
      islot[c]     : [NLOC] natural local node -> slot
      well_perm[c] : [128, CH, D] f32 edge weights, perm slot layout
      well_nat[c]  : [128, CH, D] f32 edge weights, natural slot layout
      src_nat[c]   : [D][NK128[k]] natural-padded global gather rows
      src_loc[c]   : [D][NK128[k]] (core, local) of source (for perm remap)
      NK128        : per-pass num_idxs (uniform across cores)
    """
    src = np.asarray(edge_index[0], dtype=np.int64)
    dst = np.asarray(edge_index[1], dtype=np.int64)
    w = np.asarray(edge_weight, dtype=np.float32)
    # self-loops as ordinary edges of weight 1
    loop = np.arange(N, dtype=np.int64)
    src_all = np.concatenate([loop, src])
    dst_all = np.concatenate([loop, dst])
    w_all = np.concatenate([np.ones(N, np.float32), w])

    order = np.argsort(dst_all, kind='stable')
    src_s = src_all[order]
    w_s = w_all[order]
    deg_cnt = np.bincount(dst_all, minlength=N)
    row_start = np.zeros(N + 1, np.int64)
    np.cumsum(deg_cnt, out=row_start[1:])

    perms, islots, dls = [], [], []
    for c in range(NCORES):
        dl = deg_cnt[c * NLOC:(c + 1) * NLOC]
        perm = np.argsort(-dl, kind='stable')
        islot = np.empty(NLOC, np.int64)
        islot[perm] = np.arange(NLOC)
        perms.append(perm)
        islots.append(islot)
        dls.append(dl)

    D = int(max(dl.max() for dl in dls))
    # per-pass valid count, maxed over cores, rounded to 128
    NK128 = []
    for k in range(D):
        nk = max(int((dl[perm] > k).sum()) for dl, perm in zip(dls, perms))
        if k == 0:
            nk = NLOC_PAD          # pass 0 also covers pad slots (dummy edge)
        NK128.append(max(128, ((nk + 127) // 128) * 128))

    well_perm, well_nat, src_nat, src_loc = [], [], [], []
    for c in range(NCORES):
        g0 = c * NLOC
        dl, perm = dls[c], perms[c]
        wp = np.zeros((NLOC_PAD, D), np.float32)
        wn = np.zeros((NLOC_PAD, D), np.float32)
        sn_l, sl_l = [], []
        node_of_slot = np.concatenate([g0 + perm, np.full(60, -1, np.int64)])
        deg_of_slot = np.concatenate([dl[perm], np.zeros(60, np.int64)])
        for k in range(D):
            nk = NK128[k]
            idx_nat = np.zeros(nk, np.int64)
            idx_loc = np.zeros((nk, 2), np.int64)   # (core, local)
            valid = (deg_of_slot[:nk] > k)
            vs = np.nonzero(valid)[0]
            if len(vs):
                e = row_start[node_of_slot[vs]] + k
                s = src_s[e]
                idx_nat[vs] = (s // NLOC) * NLOC_PAD + (s % NLOC)
                idx_loc[vs, 0] = s // NLOC
                idx_loc[vs, 1] = s % NLOC
                wp[vs, k] = w_s[e]
            if k == 0:
                wp[NLOC:, 0] = 1.0          # pad slots: dummy edge idx 0, w 1
            sn_l.append(idx_nat)
            sl_l.append(idx_loc)
        # natural layout weights (for deg/dinv in natural order only)
        for k in range(D):
            validn = dl > k
            wn[:NLOC, k][validn] = w_s[row_start[g0 + np.nonzero(validn)[0]] + k]
        wn[NLOC:, 0] = 1.0
        well_perm.append(np.ascontiguousarray(
            wp.reshape(CH, 128, D).transpose(1, 0, 2)))   # [128, CH, D]
        well_nat.append(np.ascontiguousarray(
            wn.reshape(CH, 128, D).transpose(1, 0, 2)))
        src_nat.append(sn_l)
        src_loc.append(sl_l)

    return dict(perms=perms, islots=islots, D=D, NK128=NK128,
                well_perm=well_perm, well_nat=well_nat,
                src_nat=src_nat, src_loc=src_loc)


def _gather_idx_tensor(gp, core, mode):
    """Concatenated wrapped gather-index tensor for one graph+conv.
    mode 'nat': rows into a natural-order [NCORES*NLOC_PAD, F] table.
    mode 'perm': rows into a perm-order table (slot positions)."""
    blocks = []
    for k in range(gp['D']):
        if mode == 'nat':
            idx = gp['src_nat'][core][k]
        else:
            loc = gp['src_loc'][core][k]
            idx = np.zeros(len(loc), np.int64)
            for r in range(NCORES):
                m = loc[:, 0] == r
                idx[m] = r * NLOC_PAD + gp['islots'][r][loc[m, 1]]
        blocks.append(_wrap_idx(idx))
    return np.concatenate(blocks, axis=1)


def _scatter_idx_tensor(gp, core):
    a = np.full(NLOC_PAD, -1, np.int64)
    a[:NLOC] = gp['perms'][core]
    return _wrap_idx(a)


def preprocess(inputs):
    """Build per-core input maps + static metadata."""
    x = np.asarray(inputs['x_RNA'], np.float32)
    xadt = np.asarray(inputs['x_ADT'], np.float32)
    gs = _graph_prep(inputs['sim_edge_index'], inputs['sim_edge_weight'])
    gd = _graph_prep(inputs['dist_edge_index'], inputs['dist_edge_weight'])
    gc = _graph_prep(inputs['common_edge_index'], inputs['common_edge_weight'])

    def pad_w(wm, kp):   # [k, n] -> [kp, n]
        w = np.zeros((kp, wm.shape[1]), np.float32)
        w[:wm.shape[0]] = wm
        return w

    w1 = pad_w(np.asarray(inputs['W_rna1'], np.float32), KPAD)
    w2 = pad_w(np.asarray(inputs['W_rna2'], np.float32), KPAD)
    wsim = np.asarray(inputs['W_sim'], np.float32)
    wdist = np.asarray(inputs['W_dist'], np.float32)
    wp3 = pad_w(np.asarray(inputs['W_p3'], np.float32), QPAD)
    wf1 = np.asarray(inputs['W_f1'], np.float32)
    wf2 = np.asarray(inputs['W_f2'], np.float32)

    def brep(b):
        return np.ascontiguousarray(
            np.broadcast_to(np.asarray(b, np.float32), (128, len(b))))

    common = dict(
        w1=w1, w2=w2, wsim=wsim, wdist=wdist, wp3=wp3, wf1=wf1, wf2=wf2,
        b1=brep(inputs['b_rna1']), b2=brep(inputs['b_rna2']),
        bsim=brep(inputs['b_sim']), bdist=brep(inputs['b_dist']),
        bp3=brep(inputs['b_p3']), bf1=brep(inputs['b_f1']),
        bf2=brep(inputs['b_f2']),
        ident=np.eye(128, dtype=np.float32),
    )

    in_maps = []
    for c in range(NCORES):
        own = slice(c * NLOC, (c + 1) * NLOC)
        xts = np.zeros((KPAD, NLOC_PAD), np.float32)
        xts[:IN_C, :NLOC] = x[own][gs['perms'][c]].T
        xtd = np.zeros((KPAD, NLOC_PAD), np.float32)
        xtd[:IN_C, :NLOC] = x[own][gd['perms'][c]].T
        xa = np.zeros((NLOC_PAD, QPAD), np.float32)
        xa[:NLOC, :Q] = xadt[own][gc['perms'][c]]   # perm_common slot order
        m = dict(common)
        m['xts'] = xts
        m['xtd'] = xtd
        m['xadt'] = xa
        for tag, gp in (('s', gs), ('d', gd), ('c', gc)):
            m[f'well_{tag}p'] = gp['well_perm'][c].reshape(128, -1)
            m[f'well_{tag}n'] = gp['well_nat'][c].reshape(128, -1)
            m[f'si_{tag}'] = _scatter_idx_tensor(gp, c)
        m['gi_s1'] = _gather_idx_tensor(gs, c, 'perm')
        m['gi_s2'] = _gather_idx_tensor(gs, c, 'perm')
        m['gi_d1'] = _gather_idx_tensor(gd, c, 'perm')
        m['gi_d2'] = _gather_idx_tensor(gd, c, 'perm')
        m['gi_c'] = _gather_idx_tensor(gc, c, 'perm')
        in_maps.append(m)

    meta = dict(gs=dict(D=gs['D'], NK128=gs['NK128']),
                gd=dict(D=gd['D'], NK128=gd['NK128']),
                gc=dict(D=gc['D'], NK128=gc['NK128']),
                perms=dict(s=gs['perms'], d=gd['perms'], c=gc['perms']))
    return in_maps, meta


# ---------------------------------------------------------------------------
# numpy emulation of the device program (for validation only)
# ---------------------------------------------------------------------------

def _np_passes(ag, well, gi, NK128, D, F):
    """ag: [NCORES*NLOC_PAD, F] gather table; well: [128, CH*D];
    gi: wrapped idx tensor. Returns acc [128, CH, F]."""
    acc = np.zeros((128, CH, F), np.float32)
    wellv = well.reshape(128, CH, D)
    col = 0
    for k in range(D):
        nk = NK128[k]
        ncol = nk // 16
        idx = gi[:16, col:col + ncol].T.reshape(-1)   # unwrap
        col += ncol
        G = ag[idx]                                    # [nk, F]
        Gs = G.reshape(nk // 128, 128, F).transpose(1, 0, 2)  # [128, blk, F]
        nblk = nk // 128
        wk = wellv[:, :nblk, k]
        acc[:, :nblk, :] += wk[:, :, None] * Gs
    return acc


def _numpy_sim(in_maps, meta):
    """Full 8-core numpy emulation; returns assembled outputs."""
    def dinv_from_well(well):
        deg = well.reshape(128, CH, -1).sum(axis=2)     # [128, CH]
        return 1.0 / np.sqrt(deg)

    def scale_vec(dinv):
        # dinv[p, ch] is for slot ch*128+p -> per-slot vector
        return dinv.T.reshape(-1)

    # conv1 matmuls + htil (natural order, padded rows)
    ag1s = np.zeros((NCORES * NLOC_PAD, HID), np.float32)
    ag1d = np.zeros((NCORES * NLOC_PAD, HID), np.float32)
    agadt = np.zeros((NCORES * NLOC_PAD, QPAD), np.float32)
    for c, m in enumerate(in_maps):
        xt = m['xt']
        h1 = xt.T @ m['w1']     # [2560, 512]
        h2 = xt.T @ m['w2']
        dsn = dinv_from_well(m['well_sn'])
        ddn = dinv_from_well(m['well_dn'])
        dcn = dinv_from_well(m['well_cn'])
        ag1s[c * NLOC_PAD:(c + 1) * NLOC_PAD] = h1 * scale_vec(dsn)[:, None]
        ag1d[c * NLOC_PAD:(c + 1) * NLOC_PAD] = h2 * scale_vec(ddn)[:, None]
        dcp = dinv_from_well(m['well_cp'])
        agadt[c * NLOC_PAD:(c + 1) * NLOC_PAD] = m['xadt'] * scale_vec(dcp)[:, None]

    outs = {k: [] for k in ['x_sim', 'x_dist', 'fused', 'fused_pro', 'pro']}
    for c, m in enumerate(in_maps):
        dsp = dinv_from_well(m['well_sp'])
        ddp = dinv_from_well(m['well_dp'])
        dcp = dinv_from_well(m['well_cp'])
        gs, gd, gc = meta['gs'], meta['gd'], meta['gc']
        acc = _np_passes(ag1s, m['well_sp'], m['gi_s1'], gs['NK128'], gs['D'], HID)
        xs = np.maximum(acc * dsp[:, :, None] + m['b1'][:, None, :], 0.0)
        acc = _np_passes(ag1d, m['well_dp'], m['gi_d1'], gd['NK128'], gd['D'], HID)
        xd = np.maximum(acc * ddp[:, :, None] + m['b2'][:, None, :], 0.0)

        # conv2 matmuls: slot-ordered rows
        def unchunk(t):   # [128, CH, F] -> [NLOC_PAD, F] slot order
            return t.transpose(1, 0, 2).reshape(NLOC_PAD, -1)
        h2s = unchunk(xs) @ m['wsim'] * unchunk(dsp[:, :, None])
        h2d = unchunk(xd) @ m['wdist'] * unchunk(ddp[:, :, None])
        outs.setdefault('_h2s', []).append(h2s)
        outs.setdefault('_h2d', []).append(h2d)

    ag2s = np.concatenate(outs.pop('_h2s'), axis=0)
    ag2d = np.concatenate(outs.pop('_h2d'), axis=0)

    for c, m in enumerate(in_maps):
        dsp = dinv_from_well(m['well_sp'])
        ddp = dinv_from_well(m['well_dp'])
        dcp = dinv_from_well(m['well_cp'])
        gs, gd, gc = meta['gs'], meta['gd'], meta['gc']
        acc = _np_passes(ag2s, m['well_sp'], m['gi_s2'], gs['NK128'], gs['D'], OUT)
        xsim = acc * dsp[:, :, None] + m['bsim'][:, None, :]
        acc = _np_passes(ag2d, m['well_dp'], m['gi_d2'], gd['NK128'], gd['D'], OUT)
        xdist = acc * ddp[:, :, None] + m['bdist'][:, None, :]
        acc = _np_passes(agadt, m['well_cp'], m['gi_c'], gc['NK128'], gc['D'], QPAD)
        accp = acc * dcp[:, :, None]
        pro = accp.transpose(1, 0, 2).reshape(NLOC_PAD, QPAD) @ m['wp3'] \
            + m['bp3'][0][None, :]

        # scatter to natural order
        def scat(t, perm):   # t [128, CH, F] slot-major
            flat = t.transpose(1, 0, 2).reshape(NLOC_PAD, -1)
            out = np.zeros((NLOC_PAD, flat.shape[1]), np.float32)
            out[perm] = flat[:NLOC]
            return out
        xsim_n = scat(xsim, meta['perms']['s'][c])
        xdist_n = scat(xdist, meta['perms']['d'][c])
        pro_n = np.zeros_like(pro)
        pro_n[meta['perms']['c'][c]] = pro[:NLOC]

        fused = np.concatenate([xsim_n, xdist_n], axis=1) @ m['wf1'] + m['bf1'][0]
        fused_pro = np.concatenate([fused, pro_n], axis=1) @ m['wf2'] + m['bf2'][0]

        outs['x_sim'].append(xsim_n[:NLOC])
        outs['x_dist'].append(xdist_n[:NLOC])
        outs['pro'].append(pro_n[:NLOC])
        outs['fused'].append(fused[:NLOC])
        outs['fused_pro'].append(fused_pro[:NLOC])

    return tuple(np.concatenate(outs[k], axis=0) for k in
                 ['x_sim', 'x_dist', 'fused', 'fused_pro', 'pro'])


# ---------------------------------------------------------------------------
# device program
# ---------------------------------------------------------------------------

def _build(meta, gdt_bf16=True):
    import concourse.bass as bass
    import concourse.bacc as bacc
    import concourse.mybir as mybir
    import concourse.tile as tile

    f32 = mybir.dt.float32
    f32r = mybir.dt.float32r
    i16 = mybir.dt.int16
    GDT = mybir.dt.bfloat16 if gdt_bf16 else f32
    MUL = mybir.AluOpType.mult
    ADD = mybir.AluOpType.add
    AFT = mybir.ActivationFunctionType
    AX = mybir.AxisListType.X

    Ds, Dd, Dc = meta['gs']['D'], meta['gd']['D'], meta['gc']['D']
    NKs, NKd, NKc = (meta['gs']['NK128'], meta['gd']['NK128'],
                     meta['gc']['NK128'])
    NALL = NCORES * NLOC_PAD

    nc = bacc.Bacc("TRN2", target_bir_lowering=False, debug=False,
                   num_devices=NCORES)

    def din(name, shape, dt=f32):
        return nc.dram_tensor(name, shape, dt, kind="ExternalInput")

    xts_in = din('xts', [KPAD, NLOC_PAD])
    xtd_in = din('xtd', [KPAD, NLOC_PAD])
    xadt_in = din('xadt', [NLOC_PAD, QPAD])
    w1_in = din('w1', [KPAD, HID])
    w2_in = din('w2', [KPAD, HID])
    wsim_in = din('wsim', [HID, OUT])
    wdist_in = din('wdist', [HID, OUT])
    wp3_in = din('wp3', [QPAD, OUT])
    wf1_in = din('wf1', [HID, OUT])
    wf2_in = din('wf2', [HID, OUT])
    bias_in = {nm: din(nm, [128, w]) for nm, w in
               (('b1', HID), ('b2', HID), ('bsim', OUT), ('bdist', OUT),
                ('bp3', OUT), ('bf1', OUT), ('bf2', OUT))}
    ident_in = din('ident', [128, 128])
    well_in = {}
    for tag, D in (('s', Ds), ('d', Dd), ('c', Dc)):
        well_in[tag + 'p'] = din(f'well_{tag}p', [128, CH * D])
        well_in[tag + 'n'] = din(f'well_{tag}n', [128, CH * D])
    gi_in = {
        's1': din('gi_s1', [128, sum(NKs) // 16], i16),
        's2': din('gi_s2', [128, sum(NKs) // 16], i16),
        'd1': din('gi_d1', [128, sum(NKd) // 16], i16),
        'd2': din('gi_d2', [128, sum(NKd) // 16], i16),
        'c': din('gi_c', [128, sum(NKc) // 16], i16),
    }
    si_in = {t: din(f'si_{t}', [128, NLOC_PAD // 16], i16) for t in 'sdc'}

    outs = {name: nc.dram_tensor(name, [NLOC_PAD, OUT], f32,
                                 kind="ExternalOutput")
            for name in ['x_sim_out', 'x_dist_out', 'fused_out',
                         'fused_pro_out', 'pro_out']}

    with tile.TileContext(nc) as tc:
        with tc.tile_pool(name="persist", bufs=1) as pp, \
             tc.tile_pool(name="gath", bufs=3) as gp, \
             tc.tile_pool(name="gidx", bufs=2) as gip, \
             tc.tile_pool(name="dram", bufs=1, space="DRAM") as dram:

            def dtile(shape, tag, shared=False, dt=None):
                return dram.tile(shape, dt or GDT, tag=tag, name=tag,
                                 addr_space="Shared" if shared else "Local")
            bnc_h1s = dtile([NLOC_PAD, HID], "bnc_h1s")
            bnc_h1d = dtile([NLOC_PAD, HID], "bnc_h1d")
            bnc_h2s = dtile([NLOC_PAD, OUT], "bnc_h2s")
            bnc_h2d = dtile([NLOC_PAD, OUT], "bnc_h2d")
            bnc_adt = dtile([NLOC_PAD, QPAD], "bnc_adt")
            ag_h1s = dtile([NALL, HID], "ag_h1s", True)
            ag_h1d = dtile([NALL, HID], "ag_h1d", True)
            ag_h2s = dtile([NALL, OUT], "ag_h2s", True)
            ag_h2d = dtile([NALL, OUT], "ag_h2d", True)
            ag_adt = dtile([NALL, QPAD], "ag_adt", True)

            def allgather(bounce, ag):
                nc.gpsimd.collective_compute(
                    "AllGather", mybir.AluOpType.bypass,
                    replica_groups=[list(range(NCORES))],
                    ins=[bounce.opt()], outs=[ag.opt()])

            # ---- persistent small tiles ----
            ident = pp.tile([128, 128], f32, tag="ident", name="ident")
            nc.sync.dma_start(ident[:], ident_in[:])
            bias = {}
            for nm, t in bias_in.items():
                w = t.shape[1]
                bias[nm] = pp.tile([128, w], f32, tag=f"bias_{nm}",
                                   name=f"bias_{nm}")
                nc.sync.dma_start(bias[nm][:], t[:])

            # pre-pin the gather pool's address space so its tiles never
            # land on P1's transient allocations (avoids false WAR waits)
            for _pi in range(3):
                _pin = gp.tile([128, SUBBLK, HID], GDT, tag="G", name="G")
                nc.vector.memset(_pin[:, :1, :1], 0)
            _pin = gp.tile([128, CH, HID], GDT, tag="Ginit", name="Ginit")
            nc.vector.memset(_pin[:, :1, :1], 0)

            # ---- P0: norm (deg -> dinv) ----
            well_sb = {}
            dinv = {}
            with tc.tile_pool(name="norm", bufs=2) as npool:
                for key, D in (('sp', Ds), ('dp', Dd), ('cp', Dc)):
                    perm_layout = key.endswith('p')
                    pool = pp if perm_layout else npool
                    wsb = pool.tile([128, CH * D], f32, tag=f"well_{key}",
                                    name=f"well_{key}")
                    nc.sync.dma_start(wsb[:], well_in[key][:])
                    if perm_layout:
                        well_sb[key[0]] = wsb
                    deg = npool.tile([128, CH], f32, tag="deg", name="deg")
                    nc.vector.tensor_reduce(
                        deg[:], wsb[:].rearrange("p (c d) -> p c d", d=D),
                        axis=AX, op=ADD)
                    rec = npool.tile([128, CH], f32, tag="rec", name="rec")
                    nc.vector.reciprocal(rec[:], deg[:])
                    dv = pp.tile([128, CH], f32, tag=f"dinv_{key}",
                                 name=f"dinv_{key}")
                    nc.scalar.activation(dv[:], rec[:], AFT.Sqrt)
                    dinv[key] = dv

            # ---- pro input scale -> bounce -> AG (early) ----
            with tc.tile_pool(name="adt", bufs=3) as ap_:
                for m in range(CH):
                    t = ap_.tile([128, QPAD], f32, tag="xa", name="xa")
                    nc.sync.dma_start(t[:], xadt_in[m * 128:(m + 1) * 128, :])
                    tg = ap_.tile([128, QPAD], GDT, tag="xag", name="xag")
                    nc.vector.tensor_scalar_mul(tg[:], t[:],
                                                dinv['cp'][:, m:m + 1])
                    nc.sync.dma_start(bnc_adt[m * 128:(m + 1) * 128, :], tg[:])
            allgather(bnc_adt, ag_adt)

            # ---- ELL pass loop: flat 8-block gather groups ----
            def run_passes(acc, ag, gi_sb, well, D, NK, F,
                           init_sbuf=None, hooks=None, group_range=None):
                """acc[p,ch,:] = sum_k well[p,ch,k] * table[src_k[slot]].

                Blocks (k, ch) are flattened k-major and gathered in groups
                of up to SUBBLK (gi columns are contiguous across passes).
                init_sbuf: optional [128, CH, F] SBUF tile holding the k=0
                (self-loop) rows in slot order - skips the k=0 gathers.
                hooks: {group_index: callable} emitted between groups (used
                to slot collective kicks / scatters into the Pool queue).
                """
                blocks = []          # (k, ch, col) with col = gi column base
                col = 0
                for k in range(D):
                    for ch in range(NK[k] // 128):
                        blocks.append((k, ch, col + ch * 8))
                    col += NK[k] // 16
                if init_sbuf is not None:
                    if group_range is None or group_range[0] == 0:
                        for ch in range(CH):
                            nc.vector.tensor_scalar_mul(
                                acc[:, ch, :], init_sbuf[:, ch, :],
                                well[:, ch * D: ch * D + 1])
                    blocks = [b for b in blocks if b[0] > 0]
                hooks = hooks or {}
                ngroups = (len(blocks) + SUBBLK - 1) // SUBBLK
                glo, ghi = group_range or (0, ngroups)
                ghi = min(ghi, ngroups)
                for g in range(glo, ghi):
                    if g in hooks:
                        hooks[g]()
                    grp = blocks[g * SUBBLK:(g + 1) * SUBBLK]
                    nb = len(grp)
                    G = gp.tile([128, SUBBLK, F], GDT, tag="G", name="G")
                    nc.gpsimd.dma_gather(
                        G[:, :nb, :], ag[:],
                        gi_sb[:, grp[0][2]: grp[0][2] + nb * 8],
                        nb * 128, nb * 128, F)
                    for i, (k, ch, _) in enumerate(grp):
                        wsl = well[:, ch * D + k: ch * D + k + 1]
                        if k == 0:
                            nc.vector.tensor_scalar_mul(
                                acc[:, ch, :], G[:, i, :], wsl)
                        else:
                            nc.vector.scalar_tensor_tensor(
                                acc[:, ch, :], G[:, i, :], wsl,
                                acc[:, ch, :], MUL, ADD)
                for cb in [hooks[g] for g in hooks if ghi == ngroups
                           and g >= ngroups]:
                    cb()

            def load_gi(name):
                t = gip.tile([128, gi_in[name].shape[1]], i16, tag="gi",
                             name=f"gi_{name}")
                nc.sync.dma_start(t[:], gi_in[name][:])
                return t

            def load_init(bnc, F):
                t = gp.tile([128, CH, F], GDT, tag="Ginit", name="Ginit")
                nc.sync.dma_start(
                    t[:], bnc[:].rearrange("(b p) f -> p b f", p=128))
                return t

            def load_si(tag):
                t = gip.tile([128, NLOC_PAD // 16], i16, tag="si",
                             name=f"si_{tag}")
                nc.sync.dma_start(t[:], si_in[tag][:])
                return t

            # ---- P1: conv1 matmuls (two W-resident passes) ----
            with tc.tile_pool(name="w12", bufs=1) as wp, \
                 tc.tile_pool(name="xt", bufs=3) as xp, \
                 tc.tile_pool(name="h1o", bufs=5) as hp, \
                 tc.tile_pool(name="psA", bufs=4, space="PSUM") as psA:
                for w_in, x_in_, dv, bnc, pst in (
                        (w1_in, xts_in, 'sp', bnc_h1s, 'ps_s'),
                        (w2_in, xtd_in, 'dp', bnc_h1d, 'ps_d')):
                    wsb = wp.tile([128, 16, HID], f32r, tag="w12",
                                  name="wsb")
                    nc.sync.dma_start(
                        wsb[:], w_in[:].rearrange("(t p) n -> p t n", p=128)
                        .bitcast(f32r))
                    for mg in range(5):
                        hgrp = hp.tile([128, 4, HID], GDT, tag="hgrp",
                                       name="hgrp")
                        pss = [psA.tile([128, HID], f32, tag=pst, name=pst)
                               for _ in range(4)]
                        for k in range(16):
                            xt_t = xp.tile([128, 512], f32r, tag="xt",
                                           name="xt_t")
                            nc.sync.dma_start(
                                xt_t[:],
                                x_in_[k * 128:(k + 1) * 128,
                                      mg * 512:(mg + 1) * 512].bitcast(f32r))
                            for mi in range(4):
                                nc.tensor.matmul(
                                    pss[mi][:],
                                    xt_t[:, mi * 128:(mi + 1) * 128],
                                    wsb[:, k, :],
                                    start=(k == 0), stop=(k == 15))
                        for mi in range(4):
                            m = mg * 4 + mi
                            nc.scalar.activation(hgrp[:, mi, :], pss[mi][:],
                                                 AFT.Copy,
                                                 scale=dinv[dv][:, m:m + 1])
                        nc.sync.dma_start(
                            bnc[mg * 512:(mg + 1) * 512, :]
                            .rearrange("(b p) f -> p b f", p=128), hgrp[:])
            # ---- pro passes (overlap P1 tail / AG_h1) ----
            pro_pool_cm = tc.tile_pool(name="pro", bufs=1)
            prop = pro_pool_cm.__enter__()
            accp_t = prop.tile([128, CH, QPAD], f32, tag="accp",
                               name="accp_t")
            gi_c = load_gi('c')
            ginit = load_init(bnc_adt, QPAD)
            run_passes(accp_t, ag_adt, gi_c, well_sb['c'], Dc, NKc, QPAD,
                       init_sbuf=ginit, group_range=(0, 24))

            allgather(bnc_h1s, ag_h1s)
            run_passes(accp_t, ag_adt, gi_c, well_sb['c'], Dc, NKc, QPAD,
                       init_sbuf=ginit, group_range=(24, 10 ** 9))
            allgather(bnc_h1d, ag_h1d)
            # ---- pro post: scale, transpose, matmul, scatter ----
            with tc.tile_pool(name="tr2", bufs=4) as trp2, \
                 tc.tile_pool(name="wsm", bufs=1) as wsp, \
                 tc.tile_pool(name="psD", bufs=2, space="PSUM") as psD, \
                 tc.tile_pool(name="psE", bufs=2, space="PSUM") as psE:
                wp3_sb = wsp.tile([128, OUT], f32r, tag="wp3", name="wp3_sb")
                nc.sync.dma_start(wp3_sb[:], wp3_in[:].bitcast(f32r))
                pro_sb = prop.tile([128, CH, OUT], f32, tag="prosb",
                                   name="pro_sb")
                for m in range(CH):
                    nc.vector.tensor_scalar_mul(
                        accp_t[:, m, :], accp_t[:, m, :],
                        dinv['cp'][:, m:m + 1])
                    tp = psD.tile([128, 128], f32, tag="tp2", name="tp2")
                    nc.tensor.transpose(tp[:], accp_t[:, m, :], ident[:])
                    xb = trp2.tile([128, 128], f32r, tag="xpT", name="xpT")
                    nc.vector.tensor_copy(xb[:], tp[:])
                    pso = psE.tile([128, OUT], f32, tag="psop", name="psop")
                    nc.tensor.matmul(pso[:], xb[:], wp3_sb[:],
                                     start=True, stop=True)
                    nc.vector.scalar_tensor_tensor(
                        pro_sb[:, m, :], pso[:], 1.0, bias['bp3'][:], MUL, ADD)
            si = load_si('c')
            nc.gpsimd.dma_scatter_add(outs['pro_out'][:], pro_sb[:],
                                      si[:], NLOC_PAD, NLOC, OUT)
            pro_pool_cm.__exit__(None, None, None)

            # ---- conv1-sim passes -> xs ----
            acc_pool_cm = tc.tile_pool(name="accbig", bufs=2)
            accpool = acc_pool_cm.__enter__()
            xs = accpool.tile([128, CH, HID], f32, tag="accbig", name="xs")
            gi = load_gi('s1')
            ginit = load_init(bnc_h1s, HID)
            run_passes(xs, ag_h1s, gi, well_sb['s'], Ds, NKs, HID,
                       init_sbuf=ginit)
            for ch in range(CH):
                nc.vector.scalar_tensor_tensor(
                    xs[:, ch, :], xs[:, ch, :], dinv['sp'][:, ch:ch + 1],
                    bias['b1'][:], MUL, ADD)
                nc.scalar.activation(xs[:, ch, :], xs[:, ch, :], AFT.Relu)

            # ---- conv2 matmul helper ----
            def conv2_mm(xsrc, wsb, dv, bnc, psB, psC, trp, hp2):
                for m in range(CH):
                    blocks = []
                    for kb in range(4):
                        tp = psB.tile([128, 128], f32, tag="tp", name="tp")
                        nc.tensor.transpose(
                            tp[:], xsrc[:, m, kb * 128:(kb + 1) * 128],
                            ident[:])
                        xb = trp.tile([128, 128], f32r, tag="xsT", name="xsT")
                        nc.vector.tensor_copy(xb[:], tp[:])
                        blocks.append(xb)
                    pso = psC.tile([128, OUT], f32, tag="pso", name="pso")
                    for kb in range(4):
                        nc.tensor.matmul(pso[:], blocks[kb][:],
                                         wsb[:, kb, :],
                                         start=(kb == 0), stop=(kb == 3))
                    h2t = hp2.tile([128, OUT], GDT, tag="h2t", name="h2t")
                    nc.scalar.activation(h2t[:], pso[:], AFT.Copy,
                                         scale=dv[:, m:m + 1])
                    nc.sync.dma_start(bnc[m * 128:(m + 1) * 128, :], h2t[:])

            _cm_w2 = tc.tile_pool(name="w2nd", bufs=1)
            _cm_tr = tc.tile_pool(name="tr", bufs=4)
            _cm_psB = tc.tile_pool(name="psB", bufs=4, space="PSUM")
            _cm_psC = tc.tile_pool(name="psC", bufs=2, space="PSUM")
            wp2 = _cm_w2.__enter__()
            trp = _cm_tr.__enter__()
            psB = _cm_psB.__enter__()
            psC = _cm_psC.__enter__()
            if True:
                wsim_sb = wp2.tile([128, 4, OUT], f32r, tag="wsim",
                                   name="wsim_sb")
                wdist_sb = wp2.tile([128, 4, OUT], f32r, tag="wdist",
                                    name="wdist_sb")
                nc.sync.dma_start(
                    wsim_sb[:], wsim_in[:].rearrange("(t p) n -> p t n", p=128)
                    .bitcast(f32r))
                nc.sync.dma_start(
                    wdist_sb[:],
                    wdist_in[:].rearrange("(t p) n -> p t n", p=128)
                    .bitcast(f32r))

                # conv2-sim mms -> bounce
                conv2_mm(xs, wsim_sb, dinv['sp'], bnc_h2s, psB, psC, trp, wp2)

                # conv1-dist passes -> xd (AG_h2s kicked mid-stream)
                xd = accpool.tile([128, CH, HID], f32, tag="accbig", name="xd")
                gi = load_gi('d1')
                ginit = load_init(bnc_h1d, HID)
                run_passes(xd, ag_h1d, gi, well_sb['d'], Dd, NKd, HID,
                           init_sbuf=ginit,
                           hooks={12: lambda: allgather(bnc_h2s, ag_h2s)})
                for ch in range(CH):
                    nc.vector.scalar_tensor_tensor(
                        xd[:, ch, :], xd[:, ch, :], dinv['dp'][:, ch:ch + 1],
                        bias['b2'][:], MUL, ADD)
                    nc.scalar.activation(xd[:, ch, :], xd[:, ch, :], AFT.Relu)

                conv2_mm(xd, wdist_sb, dinv['dp'], bnc_h2d, psB, psC, trp,
                         wp2)

            # ---- conv2-sim passes (init from SBUF self-rows) ----
            acc2_s = accpool.tile([128, CH, OUT], f32, tag="accbig",
                                  name="acc2_s")
            gi = load_gi('s2')
            ginit = load_init(bnc_h2s, OUT)
            run_passes(acc2_s, ag_h2s, gi, well_sb['s'], Ds, NKs, OUT,
                       init_sbuf=ginit,
                       hooks={12: lambda: allgather(bnc_h2d, ag_h2d)})
            for ch in range(CH):
                nc.vector.scalar_tensor_tensor(
                    acc2_s[:, ch, :], acc2_s[:, ch, :],
                    dinv['sp'][:, ch:ch + 1], bias['bsim'][:], MUL, ADD)

            def scatter_xsim():
                si = load_si('s')
                nc.gpsimd.dma_scatter_add(outs['x_sim_out'][:], acc2_s[:],
                                          si[:], NLOC_PAD, NLOC, OUT)

            # ---- conv2-dist passes ----
            acc2_d = accpool.tile([128, CH, OUT], f32, tag="accbig",
                                  name="acc2_d")
            gi = load_gi('d2')
            ginit = load_init(bnc_h2d, OUT)
            run_passes(acc2_d, ag_h2d, gi, well_sb['d'], Dd, NKd, OUT,
                       init_sbuf=ginit, hooks={6: scatter_xsim})
            for ch in range(CH):
                nc.vector.scalar_tensor_tensor(
                    acc2_d[:, ch, :], acc2_d[:, ch, :],
                    dinv['dp'][:, ch:ch + 1], bias['bdist'][:], MUL, ADD)
            si = load_si('d')
            nc.gpsimd.dma_scatter_add(outs['x_dist_out'][:], acc2_d[:],
                                      si[:], NLOC_PAD, NLOC, OUT)

            _cm_psC.__exit__(None, None, None)
            _cm_psB.__exit__(None, None, None)
            _cm_tr.__exit__(None, None, None)
            _cm_w2.__exit__(None, None, None)
            acc_pool_cm.__exit__(None, None, None)

            # ---- fused + fused_pro ----
            with tc.tile_pool(name="fus", bufs=4) as fp, \
                 tc.tile_pool(name="wf", bufs=1) as wfp, \
                 tc.tile_pool(name="trf", bufs=6) as trf, \
                 tc.tile_pool(name="psF", bufs=4, space="PSUM") as psF, \
                 tc.tile_pool(name="psG", bufs=2, space="PSUM") as psG:
                wf1_sb = wfp.tile([128, 4, OUT], f32r, tag="wf1", name="wf1_sb")
                wf2_sb = wfp.tile([128, 4, OUT], f32r, tag="wf2", name="wf2_sb")
                nc.sync.dma_start(
                    wf1_sb[:], wf1_in[:].rearrange("(t p) n -> p t n", p=128)
                    .bitcast(f32r))
                nc.sync.dma_start(
                    wf2_sb[:], wf2_in[:].rearrange("(t p) n -> p t n", p=128)
                    .bitcast(f32r))

                def tblocks(src_ap, n):
                    out = []
                    for kb in range(n):
                        tp = psF.tile([128, 128], f32, tag="tpf", name="tpf")
                        nc.tensor.transpose(
                            tp[:], src_ap[:, kb * 128:(kb + 1) * 128],
                            ident[:])
                        xb = trf.tile([128, 128], f32r, tag="fT", name="fT")
                        nc.vector.tensor_copy(xb[:], tp[:])
                        out.append(xb)
                    return out

                for m in range(CH):
                    r0, r1 = m * 128, (m + 1) * 128
                    xsn = fp.tile([128, OUT], f32, tag="xsn", name="xsn")
                    xdn = fp.tile([128, OUT], f32, tag="xdn", name="xdn")
                    nc.sync.dma_start(xsn[:], outs['x_sim_out'][r0:r1, :])
                    nc.sync.dma_start(xdn[:], outs['x_dist_out'][r0:r1, :])
                    blocks = tblocks(xsn[:], 2) + tblocks(xdn[:], 2)
                    psf = psG.tile([128, OUT], f32, tag="psf", name="psf")
                    for kb in range(4):
                        nc.tensor.matmul(psf[:], blocks[kb][:],
                                         wf1_sb[:, kb, :],
                                         start=(kb == 0), stop=(kb == 3))
                    fsd = fp.tile([128, OUT], f32, tag="fsd", name="fsd")
                    nc.vector.scalar_tensor_tensor(
                        fsd[:], psf[:], 1.0, bias['bf1'][:], MUL, ADD)
                    nc.sync.dma_start(outs['fused_out'][r0:r1, :], fsd[:])

                    prn = fp.tile([128, OUT], f32, tag="prn", name="prn")
                    nc.sync.dma_start(prn[:], outs['pro_out'][r0:r1, :])
                    blocks2 = tblocks(fsd[:], 2) + tblocks(prn[:], 2)
                    psf2 = psG.tile([128, OUT], f32, tag="psf2", name="psf2")
                    for kb in range(4):
                        nc.tensor.matmul(psf2[:], blocks2[kb][:],
                                         wf2_sb[:, kb, :],
                                         start=(kb == 0), stop=(kb == 3))
                    fpd = fp.tile([128, OUT], f32, tag="fpd", name="fpd")
                    nc.vector.scalar_tensor_tensor(
                        fpd[:], psf2[:], 1.0, bias['bf2'][:], MUL, ADD)
                    nc.sync.dma_start(outs['fused_pro_out'][r0:r1, :], fpd[:])

    nc.compile()
    return nc


_CACHE = {}
TRACE = False
LAST = {}


def kernel(**inputs):
    from concourse import bass_utils
    in_maps, meta = preprocess(inputs)
    key = (meta['gs']['D'], meta['gd']['D'], meta['gc']['D'],
           tuple(meta['gs']['NK128']), tuple(meta['gd']['NK128']),
           tuple(meta['gc']['NK128']))
    if key not in _CACHE:
        _CACHE[key] = _build(meta)
    nc = _CACHE[key]
    kw = {}
    if TRACE:
        import tempfile
        kw = dict(trace=True, tmpdir=tempfile.mkdtemp(prefix='bass_trace_'))
    res = bass_utils.run_bass_kernel_spmd(
        nc, in_maps, core_ids=list(range(NCORES)), **kw)
    if TRACE:
        LAST['exec_time_ns'] = res.exec_time_ns
        it = res.instructions_and_trace
        LAST['trace_path'] = it[1] if it else None
        LAST['tmpdir'] = kw.get('tmpdir')
    names = ['x_sim_out', 'x_dist_out', 'fused_out', 'fused_pro_out', 'pro_out']
    full = [np.concatenate([res.results[c][n][:NLOC] for c in range(NCORES)],
                           axis=0) for n in names]
    return tuple(full)



# revision 19
# speedup vs baseline: 1.2564x; 1.2564x over previous
"""DualGCN Trainium2 kernel: 8-core SPMD via bass/Tile.

Strategy (per spec sharding_hint): nodes row-sharded across 8 cores.
Each GCNConv is computed as
    agg[d] = dinv[d] * ( sum_e w_e * htil[src_e] ) + bias,   htil = dinv .* (x @ W)
with self-loops appended as ordinary edges of weight 1 (so deg = rowsum of
edge weights and the self-loop term folds into the edge sum).

Per conv: row-sharded matmul on the TensorEngine (fp32r), dinv row-scale,
AllGather of htil across the 8 cores, then an ELL-style pass loop: pass k
gathers htil[src of k-th incoming edge] for every destination slot via
dma_gather and a fused scalar_tensor_tensor accumulates acc += w_k * G_k.
Destinations are locally sorted by in-degree so pass k only covers a prefix
of slots (minimal padding). Outputs are realigned to natural row order with
a unique-index dma_scatter_add into pre-zeroed DRAM outputs.

All numerical work (matmuls, degree sums, rsqrt, scaling, aggregation) is
done on device; the host only reorders/relabels indices and pads layouts.
"""
import sys

if '/opt/trn_rl_repo' not in sys.path:
    sys.path.insert(0, '/opt/trn_rl_repo')

import numpy as np

N = 20000
NCORES = 8
NLOC = N // NCORES            # 2500
CH = 20                       # chunks of 128 per core
NLOC_PAD = CH * 128           # 2560
IN_C = 2000
KPAD = 2048
HID = 512
OUT = 256
Q = 100
QPAD = 128
SUBBLK = 8                    # gather granularity: dma_gather tops out at 1024 idxs/op
GBUFS = 6                     # in-flight gather buffers (spread over 4 SWDGE queues)
NQ = 4                        # SWDGE queues used round-robin


def _wrap_idx(a):
    """Linear int array (len % 16 == 0) -> [128, len/16] int16 SWDGE layout."""
    t = a.reshape(-1, 16).T.astype(np.int16)
    return np.ascontiguousarray(np.tile(t, (8, 1)))


def _graph_prep(edge_index, edge_weight):
    """Per-graph host-side index preprocessing.

    Returns dict with per-core ELL structures:
      perm[c]      : [NLOC] slot -> natural local node
      islot[c]     : [NLOC] natural local node -> slot
      well_perm[c] : [128, CH, D] f32 edge weights, perm slot layout
      well_nat[c]  : [128, CH, D] f32 edge weights, natural slot layout
      src_nat[c]   : [D][NK128[k]] natural-padded global gather rows
      src_loc[c]   : [D][NK128[k]] (core, local) of source (for perm remap)
      NK128        : per-pass num_idxs (uniform across cores)
    """
    src = np.asarray(edge_index[0], dtype=np.int64)
    dst = np.asarray(edge_index[1], dtype=np.int64)
    w = np.asarray(edge_weight, dtype=np.float32)
    # self-loops as ordinary edges of weight 1
    loop = np.arange(N, dtype=np.int64)
    src_all = np.concatenate([loop, src])
    dst_all = np.concatenate([loop, dst])
    w_all = np.concatenate([np.ones(N, np.float32), w])

    order = np.argsort(dst_all, kind='stable')
    src_s = src_all[order]
    w_s = w_all[order]
    deg_cnt = np.bincount(dst_all, minlength=N)
    row_start = np.zeros(N + 1, np.int64)
    np.cumsum(deg_cnt, out=row_start[1:])

    perms, islots, dls = [], [], []
    for c in range(NCORES):
        dl = deg_cnt[c * NLOC:(c + 1) * NLOC]
        perm = np.argsort(-dl, kind='stable')
        islot = np.empty(NLOC, np.int64)
        islot[perm] = np.arange(NLOC)
        perms.append(perm)
        islots.append(islot)
        dls.append(dl)

    D = int(max(dl.max() for dl in dls))
    # per-pass valid count, maxed over cores, rounded to 128
    NK128 = []
    for k in range(D):
        nk = max(int((dl[perm] > k).sum()) for dl, perm in zip(dls, perms))
        if k == 0:
            nk = NLOC_PAD          # pass 0 also covers pad slots (dummy edge)
        NK128.append(max(128, ((nk + 127) // 128) * 128))

    well_perm, well_nat, src_nat, src_loc = [], [], [], []
    for c in range(NCORES):
        g0 = c * NLOC
        dl, perm = dls[c], perms[c]
        wp = np.zeros((NLOC_PAD, D), np.float32)
        wn = np.zeros((NLOC_PAD, D), np.float32)
        sn_l, sl_l = [], []
        node_of_slot = np.concatenate([g0 + perm, np.full(60, -1, np.int64)])
        deg_of_slot = np.concatenate([dl[perm], np.zeros(60, np.int64)])
        for k in range(D):
            nk = NK128[k]
            idx_nat = np.zeros(nk, np.int64)
            idx_loc = np.zeros((nk, 2), np.int64)   # (core, local)
            valid = (deg_of_slot[:nk] > k)
            vs = np.nonzero(valid)[0]
            if len(vs):
                e = row_start[node_of_slot[vs]] + k
                s = src_s[e]
                idx_nat[vs] = (s // NLOC) * NLOC_PAD + (s % NLOC)
                idx_loc[vs, 0] = s // NLOC
                idx_loc[vs, 1] = s % NLOC
                wp[vs, k] = w_s[e]
            if k == 0:
                wp[NLOC:, 0] = 1.0          # pad slots: dummy edge idx 0, w 1
            sn_l.append(idx_nat)
            sl_l.append(idx_loc)
        # natural layout weights (for deg/dinv in natural order only)
        for k in range(D):
            validn = dl > k
            wn[:NLOC, k][validn] = w_s[row_start[g0 + np.nonzero(validn)[0]] + k]
        wn[NLOC:, 0] = 1.0
        well_perm.append(np.ascontiguousarray(
            wp.reshape(CH, 128, D).transpose(1, 0, 2)))   # [128, CH, D]
        well_nat.append(np.ascontiguousarray(
            wn.reshape(CH, 128, D).transpose(1, 0, 2)))
        src_nat.append(sn_l)
        src_loc.append(sl_l)

    return dict(perms=perms, islots=islots, D=D, NK128=NK128,
                well_perm=well_perm, well_nat=well_nat,
                src_nat=src_nat, src_loc=src_loc)


def _gather_idx_tensor(gp, core, mode):
    """Concatenated wrapped gather-index tensor for one graph+conv.
    mode 'nat': rows into a natural-order [NCORES*NLOC_PAD, F] table.
    mode 'perm': rows into a perm-order table (slot positions)."""
    blocks = []
    for k in range(gp['D']):
        if mode == 'nat':
            idx = gp['src_nat'][core][k]
        else:
            loc = gp['src_loc'][core][k]
            idx = np.zeros(len(loc), np.int64)
            for r in range(NCORES):
                m = loc[:, 0] == r
                idx[m] = r * NLOC_PAD + gp['islots'][r][loc[m, 1]]
        blocks.append(_wrap_idx(idx))
    return np.concatenate(blocks, axis=1)


def _scatter_idx_tensor(gp, core):
    a = np.full(NLOC_PAD, -1, np.int64)
    a[:NLOC] = gp['perms'][core]
    return _wrap_idx(a)


def preprocess(inputs):
    """Build per-core input maps + static metadata."""
    x = np.asarray(inputs['x_RNA'], np.float32)
    xadt = np.asarray(inputs['x_ADT'], np.float32)
    gs = _graph_prep(inputs['sim_edge_index'], inputs['sim_edge_weight'])
    gd = _graph_prep(inputs['dist_edge_index'], inputs['dist_edge_weight'])
    gc = _graph_prep(inputs['common_edge_index'], inputs['common_edge_weight'])

    def pad_w(wm, kp):   # [k, n] -> [kp, n]
        w = np.zeros((kp, wm.shape[1]), np.float32)
        w[:wm.shape[0]] = wm
        return w

    w1 = pad_w(np.asarray(inputs['W_rna1'], np.float32), KPAD)
    w2 = pad_w(np.asarray(inputs['W_rna2'], np.float32), KPAD)
    wsim = np.asarray(inputs['W_sim'], np.float32)
    wdist = np.asarray(inputs['W_dist'], np.float32)
    wp3 = pad_w(np.asarray(inputs['W_p3'], np.float32), QPAD)
    wf1 = np.asarray(inputs['W_f1'], np.float32)
    wf2 = np.asarray(inputs['W_f2'], np.float32)

    def brep(b):
        return np.ascontiguousarray(
            np.broadcast_to(np.asarray(b, np.float32), (128, len(b))))

    common = dict(
        w1=w1, w2=w2, wsim=wsim, wdist=wdist, wp3=wp3, wf1=wf1, wf2=wf2,
        b1=brep(inputs['b_rna1']), b2=brep(inputs['b_rna2']),
        bsim=brep(inputs['b_sim']), bdist=brep(inputs['b_dist']),
        bp3=brep(inputs['b_p3']), bf1=brep(inputs['b_f1']),
        bf2=brep(inputs['b_f2']),
        ident=np.eye(128, dtype=np.float32),
    )

    in_maps = []
    for c in range(NCORES):
        own = slice(c * NLOC, (c + 1) * NLOC)
        xts = np.zeros((KPAD, NLOC_PAD), np.float32)
        xts[:IN_C, :NLOC] = x[own][gs['perms'][c]].T
        xtd = np.zeros((KPAD, NLOC_PAD), np.float32)
        xtd[:IN_C, :NLOC] = x[own][gd['perms'][c]].T
        xa = np.zeros((NLOC_PAD, QPAD), np.float32)
        xa[:NLOC, :Q] = xadt[own][gc['perms'][c]]   # perm_common slot order
        m = dict(common)
        m['xts'] = xts
        m['xtd'] = xtd
        m['xadt'] = xa
        for tag, gp in (('s', gs), ('d', gd), ('c', gc)):
            m[f'well_{tag}p'] = gp['well_perm'][c].reshape(128, -1)
            m[f'well_{tag}n'] = gp['well_nat'][c].reshape(128, -1)
            m[f'si_{tag}'] = _scatter_idx_tensor(gp, c)
        m['gi_s1'] = _gather_idx_tensor(gs, c, 'perm')
        m['gi_s2'] = _gather_idx_tensor(gs, c, 'perm')
        m['gi_d1'] = _gather_idx_tensor(gd, c, 'perm')
        m['gi_d2'] = _gather_idx_tensor(gd, c, 'perm')
        m['gi_c'] = _gather_idx_tensor(gc, c, 'perm')
        in_maps.append(m)

    meta = dict(gs=dict(D=gs['D'], NK128=gs['NK128']),
                gd=dict(D=gd['D'], NK128=gd['NK128']),
                gc=dict(D=gc['D'], NK128=gc['NK128']),
                perms=dict(s=gs['perms'], d=gd['perms'], c=gc['perms']))
    return in_maps, meta


# ---------------------------------------------------------------------------
# numpy emulation of the device program (for validation only)
# ---------------------------------------------------------------------------

def _np_passes(ag, well, gi, NK128, D, F):
    """ag: [NCORES*NLOC_PAD, F] gather table; well: [128, CH*D];
    gi: wrapped idx tensor. Returns acc [128, CH, F]."""
    acc = np.zeros((128, CH, F), np.float32)
    wellv = well.reshape(128, CH, D)
    col = 0
    for k in range(D):
        nk = NK128[k]
        ncol = nk // 16
        idx = gi[:16, col:col + ncol].T.reshape(-1)   # unwrap
        col += ncol
        G = ag[idx]                                    # [nk, F]
        Gs = G.reshape(nk // 128, 128, F).transpose(1, 0, 2)  # [128, blk, F]
        nblk = nk // 128
        wk = wellv[:, :nblk, k]
        acc[:, :nblk, :] += wk[:, :, None] * Gs
    return acc


def _numpy_sim(in_maps, meta):
    """Full 8-core numpy emulation; returns assembled outputs."""
    def dinv_from_well(well):
        deg = well.reshape(128, CH, -1).sum(axis=2)     # [128, CH]
        return 1.0 / np.sqrt(deg)

    def scale_vec(dinv):
        # dinv[p, ch] is for slot ch*128+p -> per-slot vector
        return dinv.T.reshape(-1)

    # conv1 matmuls + htil (natural order, padded rows)
    ag1s = np.zeros((NCORES * NLOC_PAD, HID), np.float32)
    ag1d = np.zeros((NCORES * NLOC_PAD, HID), np.float32)
    agadt = np.zeros((NCORES * NLOC_PAD, QPAD), np.float32)
    for c, m in enumerate(in_maps):
        xt = m['xt']
        h1 = xt.T @ m['w1']     # [2560, 512]
        h2 = xt.T @ m['w2']
        dsn = dinv_from_well(m['well_sn'])
        ddn = dinv_from_well(m['well_dn'])
        dcn = dinv_from_well(m['well_cn'])
        ag1s[c * NLOC_PAD:(c + 1) * NLOC_PAD] = h1 * scale_vec(dsn)[:, None]
        ag1d[c * NLOC_PAD:(c + 1) * NLOC_PAD] = h2 * scale_vec(ddn)[:, None]
        dcp = dinv_from_well(m['well_cp'])
        agadt[c * NLOC_PAD:(c + 1) * NLOC_PAD] = m['xadt'] * scale_vec(dcp)[:, None]

    outs = {k: [] for k in ['x_sim', 'x_dist', 'fused', 'fused_pro', 'pro']}
    for c, m in enumerate(in_maps):
        dsp = dinv_from_well(m['well_sp'])
        ddp = dinv_from_well(m['well_dp'])
        dcp = dinv_from_well(m['well_cp'])
        gs, gd, gc = meta['gs'], meta['gd'], meta['gc']
        acc = _np_passes(ag1s, m['well_sp'], m['gi_s1'], gs['NK128'], gs['D'], HID)
        xs = np.maximum(acc * dsp[:, :, None] + m['b1'][:, None, :], 0.0)
        acc = _np_passes(ag1d, m['well_dp'], m['gi_d1'], gd['NK128'], gd['D'], HID)
        xd = np.maximum(acc * ddp[:, :, None] + m['b2'][:, None, :], 0.0)

        # conv2 matmuls: slot-ordered rows
        def unchunk(t):   # [128, CH, F] -> [NLOC_PAD, F] slot order
            return t.transpose(1, 0, 2).reshape(NLOC_PAD, -1)
        h2s = unchunk(xs) @ m['wsim'] * unchunk(dsp[:, :, None])
        h2d = unchunk(xd) @ m['wdist'] * unchunk(ddp[:, :, None])
        outs.setdefault('_h2s', []).append(h2s)
        outs.setdefault('_h2d', []).append(h2d)

    ag2s = np.concatenate(outs.pop('_h2s'), axis=0)
    ag2d = np.concatenate(outs.pop('_h2d'), axis=0)

    for c, m in enumerate(in_maps):
        dsp = dinv_from_well(m['well_sp'])
        ddp = dinv_from_well(m['well_dp'])
        dcp = dinv_from_well(m['well_cp'])
        gs, gd, gc = meta['gs'], meta['gd'], meta['gc']
        acc = _np_passes(ag2s, m['well_sp'], m['gi_s2'], gs['NK128'], gs['D'], OUT)
        xsim = acc * dsp[:, :, None] + m['bsim'][:, None, :]
        acc = _np_passes(ag2d, m['well_dp'], m['gi_d2'], gd['NK128'], gd['D'], OUT)
        xdist = acc * ddp[:, :, None] + m['bdist'][:, None, :]
        acc = _np_passes(agadt, m['well_cp'], m['gi_c'], gc['NK128'], gc['D'], QPAD)
        accp = acc * dcp[:, :, None]
        pro = accp.transpose(1, 0, 2).reshape(NLOC_PAD, QPAD) @ m['wp3'] \
            + m['bp3'][0][None, :]

        # scatter to natural order
        def scat(t, perm):   # t [128, CH, F] slot-major
            flat = t.transpose(1, 0, 2).reshape(NLOC_PAD, -1)
            out = np.zeros((NLOC_PAD, flat.shape[1]), np.float32)
            out[perm] = flat[:NLOC]
            return out
        xsim_n = scat(xsim, meta['perms']['s'][c])
        xdist_n = scat(xdist, meta['perms']['d'][c])
        pro_n = np.zeros_like(pro)
        pro_n[meta['perms']['c'][c]] = pro[:NLOC]

        fused = np.concatenate([xsim_n, xdist_n], axis=1) @ m['wf1'] + m['bf1'][0]
        fused_pro = np.concatenate([fused, pro_n], axis=1) @ m['wf2'] + m['bf2'][0]

        outs['x_sim'].append(xsim_n[:NLOC])
        outs['x_dist'].append(xdist_n[:NLOC])
        outs['pro'].append(pro_n[:NLOC])
        outs['fused'].append(fused[:NLOC])
        outs['fused_pro'].append(fused_pro[:NLOC])

    return tuple(np.concatenate(outs[k], axis=0) for k in
                 ['x_sim', 'x_dist', 'fused', 'fused_pro', 'pro'])


# ---------------------------------------------------------------------------
# device program
# ---------------------------------------------------------------------------

def _build(meta, gdt_bf16=True):
    import concourse.bass as bass
    import concourse.bacc as bacc
    import concourse.mybir as mybir
    import concourse.tile as tile

    f32 = mybir.dt.float32
    f32r = mybir.dt.float32r
    i16 = mybir.dt.int16
    GDT = mybir.dt.bfloat16 if gdt_bf16 else f32
    MUL = mybir.AluOpType.mult
    ADD = mybir.AluOpType.add
    AFT = mybir.ActivationFunctionType
    AX = mybir.AxisListType.X

    Ds, Dd, Dc = meta['gs']['D'], meta['gd']['D'], meta['gc']['D']
    NKs, NKd, NKc = (meta['gs']['NK128'], meta['gd']['NK128'],
                     meta['gc']['NK128'])
    NALL = NCORES * NLOC_PAD

    nc = bacc.Bacc("TRN2", target_bir_lowering=False, debug=False,
                   num_devices=NCORES, num_swdge_queues=4)

    def din(name, shape, dt=f32):
        return nc.dram_tensor(name, shape, dt, kind="ExternalInput")

    xts_in = din('xts', [KPAD, NLOC_PAD])
    xtd_in = din('xtd', [KPAD, NLOC_PAD])
    xadt_in = din('xadt', [NLOC_PAD, QPAD])
    w1_in = din('w1', [KPAD, HID])
    w2_in = din('w2', [KPAD, HID])
    wsim_in = din('wsim', [HID, OUT])
    wdist_in = din('wdist', [HID, OUT])
    wp3_in = din('wp3', [QPAD, OUT])
    wf1_in = din('wf1', [HID, OUT])
    wf2_in = din('wf2', [HID, OUT])
    bias_in = {nm: din(nm, [128, w]) for nm, w in
               (('b1', HID), ('b2', HID), ('bsim', OUT), ('bdist', OUT),
                ('bp3', OUT), ('bf1', OUT), ('bf2', OUT))}
    ident_in = din('ident', [128, 128])
    well_in = {}
    for tag, D in (('s', Ds), ('d', Dd), ('c', Dc)):
        well_in[tag + 'p'] = din(f'well_{tag}p', [128, CH * D])
        well_in[tag + 'n'] = din(f'well_{tag}n', [128, CH * D])
    gi_in = {
        's1': din('gi_s1', [128, sum(NKs) // 16], i16),
        's2': din('gi_s2', [128, sum(NKs) // 16], i16),
        'd1': din('gi_d1', [128, sum(NKd) // 16], i16),
        'd2': din('gi_d2', [128, sum(NKd) // 16], i16),
        'c': din('gi_c', [128, sum(NKc) // 16], i16),
    }
    si_in = {t: din(f'si_{t}', [128, NLOC_PAD // 16], i16) for t in 'sdc'}

    outs = {name: nc.dram_tensor(name, [NLOC_PAD, OUT], f32,
                                 kind="ExternalOutput")
            for name in ['x_sim_out', 'x_dist_out', 'fused_out',
                         'fused_pro_out', 'pro_out']}

    with tile.TileContext(nc) as tc:
        with tc.tile_pool(name="persist", bufs=1) as pp, \
             tc.tile_pool(name="gath", bufs=GBUFS) as gp, \
             tc.tile_pool(name="gidx", bufs=2) as gip, \
             tc.tile_pool(name="dram", bufs=1, space="DRAM") as dram:

            def dtile(shape, tag, shared=False, dt=None):
                return dram.tile(shape, dt or GDT, tag=tag, name=tag,
                                 addr_space="Shared" if shared else "Local")
            bnc_h1s = dtile([NLOC_PAD, HID], "bnc_h1s")
            bnc_h1d = dtile([NLOC_PAD, HID], "bnc_h1d")
            bnc_h2s = dtile([NLOC_PAD, OUT], "bnc_h2s")
            bnc_h2d = dtile([NLOC_PAD, OUT], "bnc_h2d")
            bnc_adt = dtile([NLOC_PAD, QPAD], "bnc_adt")
            ag_h1s = dtile([NALL, HID], "ag_h1s", True)
            ag_h1d = dtile([NALL, HID], "ag_h1d", True)
            ag_h2s = dtile([NALL, OUT], "ag_h2s", True)
            ag_h2d = dtile([NALL, OUT], "ag_h2d", True)
            ag_adt = dtile([NALL, QPAD], "ag_adt", True)

            def allgather(bounce, ag):
                nc.gpsimd.collective_compute(
                    "AllGather", mybir.AluOpType.bypass,
                    replica_groups=[list(range(NCORES))],
                    ins=[bounce.opt()], outs=[ag.opt()])

            # ---- persistent small tiles ----
            ident = pp.tile([128, 128], f32, tag="ident", name="ident")
            nc.sync.dma_start(ident[:], ident_in[:])
            bias = {}
            for nm, t in bias_in.items():
                w = t.shape[1]
                bias[nm] = pp.tile([128, w], f32, tag=f"bias_{nm}",
                                   name=f"bias_{nm}")
                nc.sync.dma_start(bias[nm][:], t[:])

            # pre-pin the gather pool's address space so its tiles never
            # land on P1's transient allocations (avoids false WAR waits)
            for _pi in range(GBUFS):
                _pin = gp.tile([128, SUBBLK, HID], GDT, tag="G", name="G")
                nc.vector.memset(_pin[:, :1, :1], 0)
            _pin = gp.tile([128, CH, HID], GDT, tag="Ginit", name="Ginit",
                           bufs=1)
            nc.vector.memset(_pin[:, :1, :1], 0)

            # ---- P0: norm (deg -> dinv) ----
            well_sb = {}
            dinv = {}
            with tc.tile_pool(name="norm", bufs=2) as npool:
                for key, D in (('sp', Ds), ('dp', Dd), ('cp', Dc)):
                    perm_layout = key.endswith('p')
                    pool = pp if perm_layout else npool
                    wsb = pool.tile([128, CH * D], f32, tag=f"well_{key}",
                                    name=f"well_{key}")
                    nc.sync.dma_start(wsb[:], well_in[key][:])
                    if perm_layout:
                        well_sb[key[0]] = wsb
                    deg = npool.tile([128, CH], f32, tag="deg", name="deg")
                    nc.vector.tensor_reduce(
                        deg[:], wsb[:].rearrange("p (c d) -> p c d", d=D),
                        axis=AX, op=ADD)
                    rec = npool.tile([128, CH], f32, tag="rec", name="rec")
                    nc.vector.reciprocal(rec[:], deg[:])
                    dv = pp.tile([128, CH], f32, tag=f"dinv_{key}",
                                 name=f"dinv_{key}")
                    nc.scalar.activation(dv[:], rec[:], AFT.Sqrt)
                    dinv[key] = dv

            # ---- pro input scale -> bounce -> AG (early) ----
            with tc.tile_pool(name="adt", bufs=3) as ap_:
                for m in range(CH):
                    t = ap_.tile([128, QPAD], f32, tag="xa", name="xa")
                    nc.sync.dma_start(t[:], xadt_in[m * 128:(m + 1) * 128, :])
                    tg = ap_.tile([128, QPAD], GDT, tag="xag", name="xag")
                    nc.vector.tensor_scalar_mul(tg[:], t[:],
                                                dinv['cp'][:, m:m + 1])
                    nc.sync.dma_start(bnc_adt[m * 128:(m + 1) * 128, :], tg[:])
            allgather(bnc_adt, ag_adt)

            # ---- ELL pass loop: flat 8-block gather groups ----
            def run_passes(acc, ag, gi_sb, well, D, NK, F,
                           init_sbuf=None, hooks=None, group_range=None):
                """acc[p,ch,:] = sum_k well[p,ch,k] * table[src_k[slot]].

                Blocks (k, ch) are flattened k-major and gathered in groups
                of up to SUBBLK (gi columns are contiguous across passes).
                init_sbuf: optional [128, CH, F] SBUF tile holding the k=0
                (self-loop) rows in slot order - skips the k=0 gathers.
                hooks: {group_index: callable} emitted between groups (used
                to slot collective kicks / scatters into the Pool queue).
                """
                blocks = []          # (k, ch, col) with col = gi column base
                col = 0
                for k in range(D):
                    for ch in range(NK[k] // 128):
                        blocks.append((k, ch, col + ch * 8))
                    col += NK[k] // 16
                # walrus rejects TensorScalarPtr on Pool, so accumulates all
                # run on DVE (STT has no fast modes; ~1 col/cycle is the floor)
                def eng(ch):
                    return nc.vector
                if init_sbuf is not None:
                    if group_range is None or group_range[0] == 0:
                        for ch in range(CH):
                            eng(ch).tensor_scalar_mul(
                                acc[:, ch, :], init_sbuf[:, ch, :],
                                well[:, ch * D: ch * D + 1])
                    blocks = [b for b in blocks if b[0] > 0]
                hooks = hooks or {}
                ngroups = (len(blocks) + SUBBLK - 1) // SUBBLK
                glo, ghi = group_range or (0, ngroups)
                ghi = min(ghi, ngroups)
                for g in range(glo, ghi):
                    if g in hooks:
                        hooks[g]()
                    grp = blocks[g * SUBBLK:(g + 1) * SUBBLK]
                    nb = len(grp)
                    G = gp.tile([128, SUBBLK, F], GDT, tag="G", name="G")
                    nc.gpsimd.dma_gather(
                        G[:, :nb, :], ag[:],
                        gi_sb[:, grp[0][2]: grp[0][2] + nb * 8],
                        nb * 128, nb * 128, F, queue_num=g % NQ)
                    for i, (k, ch, _) in enumerate(grp):
                        wsl = well[:, ch * D + k: ch * D + k + 1]
                        if k == 0:
                            eng(ch).tensor_scalar_mul(
                                acc[:, ch, :], G[:, i, :], wsl)
                        else:
                            eng(ch).scalar_tensor_tensor(
                                acc[:, ch, :], G[:, i, :], wsl,
                                acc[:, ch, :], MUL, ADD)
                for cb in [hooks[g] for g in hooks if ghi == ngroups
                           and g >= ngroups]:
                    cb()

            def load_gi(name):
                t = gip.tile([128, gi_in[name].shape[1]], i16, tag="gi",
                             name=f"gi_{name}")
                nc.sync.dma_start(t[:], gi_in[name][:])
                return t

            def load_init(bnc, F):
                t = gp.tile([128, CH, F], GDT, tag="Ginit", name="Ginit",
                            bufs=1)
                nc.sync.dma_start(
                    t[:], bnc[:].rearrange("(b p) f -> p b f", p=128))
                return t

            def load_si(tag):
                t = gip.tile([128, NLOC_PAD // 16], i16, tag="si",
                             name=f"si_{tag}")
                nc.sync.dma_start(t[:], si_in[tag][:])
                return t

            # ---- P1: conv1 matmuls (two W-resident passes). The sim pass
            # kicks AG_h1s immediately so the collective overlaps the dist
            # matmuls and the pro pass loop; AG_h1d overlaps pro post. ----
            pro_pool_cm = tc.tile_pool(name="pro", bufs=1)
            prop = pro_pool_cm.__enter__()
            with tc.tile_pool(name="w12", bufs=1) as wp, \
                 tc.tile_pool(name="xt", bufs=3) as xp, \
                 tc.tile_pool(name="h1o", bufs=5) as hp, \
                 tc.tile_pool(name="psA", bufs=4, space="PSUM") as psA:

                def conv1_mm(w_in, x_in_, dv, bnc, pst):
                    wsb = wp.tile([128, 16, HID], f32r, tag="w12",
                                  name="wsb")
                    nc.sync.dma_start(
                        wsb[:], w_in[:].rearrange("(t p) n -> p t n", p=128)
                        .bitcast(f32r))
                    for mg in range(5):
                        hgrp = hp.tile([128, 4, HID], GDT, tag="hgrp",
                                       name="hgrp")
                        pss = [psA.tile([128, HID], f32, tag=pst, name=pst)
                               for _ in range(4)]
                        for k in range(16):
                            xt_t = xp.tile([128, 512], f32r, tag="xt",
                                           name="xt_t")
                            nc.sync.dma_start(
                                xt_t[:],
                                x_in_[k * 128:(k + 1) * 128,
                                      mg * 512:(mg + 1) * 512].bitcast(f32r))
                            for mi in range(4):
                                nc.tensor.matmul(
                                    pss[mi][:],
                                    xt_t[:, mi * 128:(mi + 1) * 128],
                                    wsb[:, k, :],
                                    start=(k == 0), stop=(k == 15))
                        for mi in range(4):
                            m = mg * 4 + mi
                            nc.scalar.activation(hgrp[:, mi, :], pss[mi][:],
                                                 AFT.Copy,
                                                 scale=dinv[dv][:, m:m + 1])
                        nc.sync.dma_start(
                            bnc[mg * 512:(mg + 1) * 512, :]
                            .rearrange("(b p) f -> p b f", p=128), hgrp[:])

                conv1_mm(w1_in, xts_in, 'sp', bnc_h1s, 'ps_s')
                allgather(bnc_h1s, ag_h1s)

                # ---- pro passes (Pool/DVE) overlap the dist matmuls (PE)
                accp_t = prop.tile([128, CH, QPAD], f32, tag="accp",
                                   name="accp_t")
                gi_c = load_gi('c')
                ginit = load_init(bnc_adt, QPAD)
                run_passes(accp_t, ag_adt, gi_c, well_sb['c'], Dc, NKc, QPAD,
                           init_sbuf=ginit)

                conv1_mm(w2_in, xtd_in, 'dp', bnc_h1d, 'ps_d')
                allgather(bnc_h1d, ag_h1d)

            # ---- pro post: scale, transpose, matmul, scatter ----
            with tc.tile_pool(name="tr2", bufs=4) as trp2, \
                 tc.tile_pool(name="wsm", bufs=1) as wsp, \
                 tc.tile_pool(name="psD", bufs=2, space="PSUM") as psD, \
                 tc.tile_pool(name="psE", bufs=2, space="PSUM") as psE:
                wp3_sb = wsp.tile([128, OUT], f32r, tag="wp3", name="wp3_sb")
                nc.sync.dma_start(wp3_sb[:], wp3_in[:].bitcast(f32r))
                pro_sb = prop.tile([128, CH, OUT], f32, tag="prosb",
                                   name="pro_sb")
                for m in range(CH):
                    nc.vector.tensor_scalar_mul(
                        accp_t[:, m, :], accp_t[:, m, :],
                        dinv['cp'][:, m:m + 1])
                    tp = psD.tile([128, 128], f32, tag="tp2", name="tp2")
                    nc.tensor.transpose(tp[:], accp_t[:, m, :], ident[:])
                    xb = trp2.tile([128, 128], f32r, tag="xpT", name="xpT")
                    nc.vector.tensor_copy(xb[:], tp[:])
                    pso = psE.tile([128, OUT], f32, tag="psop", name="psop")
                    nc.tensor.matmul(pso[:], xb[:], wp3_sb[:],
                                     start=True, stop=True)
                    nc.vector.scalar_tensor_tensor(
                        pro_sb[:, m, :], pso[:], 1.0, bias['bp3'][:], MUL, ADD)
            si = load_si('c')
            nc.gpsimd.dma_scatter_add(outs['pro_out'][:], pro_sb[:],
                                      si[:], NLOC_PAD, NLOC, OUT,
                                      queue_num=3)
            pro_pool_cm.__exit__(None, None, None)

            # ---- conv1-sim passes -> xs ----
            acc_pool_cm = tc.tile_pool(name="accbig", bufs=2)
            accpool = acc_pool_cm.__enter__()
            xs = accpool.tile([128, CH, HID], f32, tag="accbig", name="xs")
            gi = load_gi('s1')
            ginit = load_init(bnc_h1s, HID)
            run_passes(xs, ag_h1s, gi, well_sb['s'], Ds, NKs, HID,
                       init_sbuf=ginit)
            for ch in range(CH):
                nc.vector.scalar_tensor_tensor(
                    xs[:, ch, :], xs[:, ch, :], dinv['sp'][:, ch:ch + 1],
                    bias['b1'][:], MUL, ADD)
                nc.scalar.activation(xs[:, ch, :], xs[:, ch, :], AFT.Relu)

            # ---- conv2 matmul helper ----
            def conv2_mm(xsrc, wsb, dv, bnc, psB, psC, trp, hp2):
                for m in range(CH):
                    blocks = []
                    for kb in range(4):
                        tp = psB.tile([128, 128], f32, tag="tp", name="tp")
                        nc.tensor.transpose(
                            tp[:], xsrc[:, m, kb * 128:(kb + 1) * 128],
                            ident[:])
                        xb = trp.tile([128, 128], f32r, tag="xsT", name="xsT")
                        nc.vector.tensor_copy(xb[:], tp[:])
                        blocks.append(xb)
                    pso = psC.tile([128, OUT], f32, tag="pso", name="pso")
                    for kb in range(4):
                        nc.tensor.matmul(pso[:], blocks[kb][:],
                                         wsb[:, kb, :],
                                         start=(kb == 0), stop=(kb == 3))
                    h2t = hp2.tile([128, OUT], GDT, tag="h2t", name="h2t")
                    nc.scalar.activation(h2t[:], pso[:], AFT.Copy,
                                         scale=dv[:, m:m + 1])
                    nc.sync.dma_start(bnc[m * 128:(m + 1) * 128, :], h2t[:])

            _cm_w2 = tc.tile_pool(name="w2nd", bufs=1)
            _cm_tr = tc.tile_pool(name="tr", bufs=4)
            _cm_psB = tc.tile_pool(name="psB", bufs=4, space="PSUM")
            _cm_psC = tc.tile_pool(name="psC", bufs=2, space="PSUM")
            wp2 = _cm_w2.__enter__()
            trp = _cm_tr.__enter__()
            psB = _cm_psB.__enter__()
            psC = _cm_psC.__enter__()
            if True:
                wsim_sb = wp2.tile([128, 4, OUT], f32r, tag="wsim",
                                   name="wsim_sb")
                wdist_sb = wp2.tile([128, 4, OUT], f32r, tag="wdist",
                                    name="wdist_sb")
                nc.sync.dma_start(
                    wsim_sb[:], wsim_in[:].rearrange("(t p) n -> p t n", p=128)
                    .bitcast(f32r))
                nc.sync.dma_start(
                    wdist_sb[:],
                    wdist_in[:].rearrange("(t p) n -> p t n", p=128)
                    .bitcast(f32r))

                # conv2-sim mms -> bounce
                conv2_mm(xs, wsim_sb, dinv['sp'], bnc_h2s, psB, psC, trp, wp2)

                # conv1-dist passes -> xd (AG_h2s kicked mid-stream)
                xd = accpool.tile([128, CH, HID], f32, tag="accbig", name="xd")
                gi = load_gi('d1')
                ginit = load_init(bnc_h1d, HID)
                run_passes(xd, ag_h1d, gi, well_sb['d'], Dd, NKd, HID,
                           init_sbuf=ginit,
                           hooks={2: lambda: allgather(bnc_h2s, ag_h2s)})
                for ch in range(CH):
                    nc.vector.scalar_tensor_tensor(
                        xd[:, ch, :], xd[:, ch, :], dinv['dp'][:, ch:ch + 1],
                        bias['b2'][:], MUL, ADD)
                    nc.scalar.activation(xd[:, ch, :], xd[:, ch, :], AFT.Relu)

                conv2_mm(xd, wdist_sb, dinv['dp'], bnc_h2d, psB, psC, trp,
                         wp2)

            # ---- conv2-sim passes (init from SBUF self-rows) ----
            acc2_s = accpool.tile([128, CH, OUT], f32, tag="accbig",
                                  name="acc2_s")
            gi = load_gi('s2')
            ginit = load_init(bnc_h2s, OUT)
            run_passes(acc2_s, ag_h2s, gi, well_sb['s'], Ds, NKs, OUT,
                       init_sbuf=ginit,
                       hooks={2: lambda: allgather(bnc_h2d, ag_h2d)})
            for ch in range(CH):
                nc.vector.scalar_tensor_tensor(
                    acc2_s[:, ch, :], acc2_s[:, ch, :],
                    dinv['sp'][:, ch:ch + 1], bias['bsim'][:], MUL, ADD)

            def scatter_xsim():
                si = load_si('s')
                nc.gpsimd.dma_scatter_add(outs['x_sim_out'][:], acc2_s[:],
                                          si[:], NLOC_PAD, NLOC, OUT,
                                          queue_num=3)

            # ---- conv2-dist passes ----
            acc2_d = accpool.tile([128, CH, OUT], f32, tag="accbig",
                                  name="acc2_d")
            gi = load_gi('d2')
            ginit = load_init(bnc_h2d, OUT)
            run_passes(acc2_d, ag_h2d, gi, well_sb['d'], Dd, NKd, OUT,
                       init_sbuf=ginit, hooks={6: scatter_xsim})
            for ch in range(CH):
                nc.vector.scalar_tensor_tensor(
                    acc2_d[:, ch, :], acc2_d[:, ch, :],
                    dinv['dp'][:, ch:ch + 1], bias['bdist'][:], MUL, ADD)
            si = load_si('d')
            nc.gpsimd.dma_scatter_add(outs['x_dist_out'][:], acc2_d[:],
                                      si[:], NLOC_PAD, NLOC, OUT,
                                      queue_num=3)

            _cm_psC.__exit__(None, None, None)
            _cm_psB.__exit__(None, None, None)
            _cm_tr.__exit__(None, None, None)
            _cm_w2.__exit__(None, None, None)
            acc_pool_cm.__exit__(None, None, None)

            # ---- fused + fused_pro ----
            with tc.tile_pool(name="fus", bufs=4) as fp, \
                 tc.tile_pool(name="wf", bufs=1) as wfp, \
                 tc.tile_pool(name="trf", bufs=6) as trf, \
                 tc.tile_pool(name="psF", bufs=4, space="PSUM") as psF, \
                 tc.tile_pool(name="psG", bufs=2, space="PSUM") as psG:
                wf1_sb = wfp.tile([128, 4, OUT], f32r, tag="wf1", name="wf1_sb")
                wf2_sb = wfp.tile([128, 4, OUT], f32r, tag="wf2", name="wf2_sb")
                nc.sync.dma_start(
                    wf1_sb[:], wf1_in[:].rearrange("(t p) n -> p t n", p=128)
                    .bitcast(f32r))
                nc.sync.dma_start(
                    wf2_sb[:], wf2_in[:].rearrange("(t p) n -> p t n", p=128)
                    .bitcast(f32r))

                def tblocks(src_ap, n):
                    out = []
                    for kb in range(n):
                        tp = psF.tile([128, 128], f32, tag="tpf", name="tpf")
                        nc.tensor.transpose(
                            tp[:], src_ap[:, kb * 128:(kb + 1) * 128],
                            ident[:])
                        xb = trf.tile([128, 128], f32r, tag="fT", name="fT")
                        nc.vector.tensor_copy(xb[:], tp[:])
                        out.append(xb)
                    return out

                for m in range(CH):
                    r0, r1 = m * 128, (m + 1) * 128
                    xsn = fp.tile([128, OUT], f32, tag="xsn", name="xsn")
                    xdn = fp.tile([128, OUT], f32, tag="xdn", name="xdn")
                    nc.sync.dma_start(xsn[:], outs['x_sim_out'][r0:r1, :])
                    nc.sync.dma_start(xdn[:], outs['x_dist_out'][r0:r1, :])
                    blocks = tblocks(xsn[:], 2) + tblocks(xdn[:], 2)
                    psf = psG.tile([128, OUT], f32, tag="psf", name="psf")
                    for kb in range(4):
                        nc.tensor.matmul(psf[:], blocks[kb][:],
                                         wf1_sb[:, kb, :],
                                         start=(kb == 0), stop=(kb == 3))
                    fsd = fp.tile([128, OUT], f32, tag="fsd", name="fsd")
                    nc.vector.scalar_tensor_tensor(
                        fsd[:], psf[:], 1.0, bias['bf1'][:], MUL, ADD)
                    nc.sync.dma_start(outs['fused_out'][r0:r1, :], fsd[:])

                    prn = fp.tile([128, OUT], f32, tag="prn", name="prn")
                    nc.sync.dma_start(prn[:], outs['pro_out'][r0:r1, :])
                    blocks2 = tblocks(fsd[:], 2) + tblocks(prn[:], 2)
                    psf2 = psG.tile([128, OUT], f32, tag="psf2", name="psf2")
                    for kb in range(4):
                        nc.tensor.matmul(psf2[:], blocks2[kb][:],
                                         wf2_sb[:, kb, :],
                                         start=(kb == 0), stop=(kb == 3))
                    fpd = fp.tile([128, OUT], f32, tag="fpd", name="fpd")
                    nc.vector.scalar_tensor_tensor(
                        fpd[:], psf2[:], 1.0, bias['bf2'][:], MUL, ADD)
                    nc.sync.dma_start(outs['fused_pro_out'][r0:r1, :], fpd[:])

    nc.compile()
    return nc


_CACHE = {}
TRACE = False
LAST = {}


def kernel(**inputs):
    from concourse import bass_utils
    in_maps, meta = preprocess(inputs)
    key = (meta['gs']['D'], meta['gd']['D'], meta['gc']['D'],
           tuple(meta['gs']['NK128']), tuple(meta['gd']['NK128']),
           tuple(meta['gc']['NK128']))
    if key not in _CACHE:
        _CACHE[key] = _build(meta)
    nc = _CACHE[key]
    kw = {}
    if TRACE:
        import tempfile
        kw = dict(trace=True, tmpdir=tempfile.mkdtemp(prefix='bass_trace_'))
    res = bass_utils.run_bass_kernel_spmd(
        nc, in_maps, core_ids=list(range(NCORES)), **kw)
    if TRACE:
        LAST['exec_time_ns'] = res.exec_time_ns
        it = res.instructions_and_trace
        LAST['trace_path'] = it[1] if it else None
        LAST['tmpdir'] = kw.get('tmpdir')
    names = ['x_sim_out', 'x_dist_out', 'fused_out', 'fused_pro_out', 'pro_out']
    full = [np.concatenate([res.results[c][n][:NLOC] for c in range(NCORES)],
                           axis=0) for n in names]
    return tuple(full)

